# revision 119
# baseline (speedup 1.0000x reference)
"""Trainium2 Bass kernel for nn_Attention_45749991637079.

Reference computation (per batch b, C=192 channels, 128x128 image, 8 heads):
    qkv  = w_qkv @ x                       (1x1 conv; k-branch unused)
    q,v  = depthwise 3x3 (SAME) of the q/v channel blocks
    qd   = q[:, ::2, ::2]                  (64x64 downsample)
    attn = softmax(l2norm-rows(qd_h) gram * temp)   per head (24x24)
    out  = w_proj @ (attn @ v)             == (w_proj @ blockdiag(attn)) @ v

Sharding: data-parallel over batch; one batch per NeuronCore (8 cores).

Final design notes (measured ~245us on HW vs 1028us baseline; PE ~86%
busy with all major streams at the ~213ns/512-col floor):
  - fp8e4m3 on the q path only (l2norm + the gram's 4096-pixel sum damp
    quantization noise; v path stays f16): q0 taps = 3 DoubleRow
    col-pair streams (k-tile dim = low bit of the padded-buffer column)
    + row-pair + single; q1 taps = 3 three-tap DoubleRow streams
    (partition-stacked dup|orig x dj windows; dup shifted LEFT so the
    window base byte stays even -- odd bases fault the DoubleRow
    ifmap fetch).
  - v1 taps: 5 streams/subtile via TWO dup planes (col-shift pairs
    (di,0)+(di,1); row-shift plane pairs (1,2)+(0,2)).
  - pw stage copies issued AFTER qtap so the qd stages and qdT copies
    are not stuck behind them in the in-order ACT/DVE queues (removed a
    1.8us/band PSUM-ring stall on the gram transposes).
  - overlap-save pointwise: each image row's 1x1 conv computed exactly once;
    2 halo rows copied from the previous band's padded buffer.
  - chunk1 (64-ch) tap pairing: a flat-shifted duplicate of the chunk1
    plane (one contiguous SBUF->SBUF DMA per band) lets one 128-row
    matmul apply TWO depthwise taps; 9 taps -> 6 streams per subtile.
  - gram transposes issued a band early, decoupled from gram matmuls.
  - v-taps of bands 5..7 deferred until after the softmax chain is
    issued, so the PE chews taps while ACT/DVE/DMA run the middle
    (keeps the PE clock high through the final sweep).
  - masked softmax: per-head max/sum via 0/1 masks, blockdiag(attn) via
    elementwise mask multiply -- no small extract/scatter DMAs.
  - same-tile-config matmuls grouped (PE reconfiguration costs ~110ns),
    and every K=64 contraction zero-padded to K=128 (zero data rows x
    finite weights contribute 0): K=64-vs-M=128 streams measured ~100ns
    slower per 512 cols than uniform (128,*) streams.
  - f16 output (host upcasts): halves the out-DMA bytes; consts coalesced
    into two pack DMAs; x bands prefetched one band ahead.
"""

import numpy as np

C = 192
H = W = 128
HW = H * W
HEADS = 8
CHD = 24
P0, P1 = 128, 64          # channel partition chunks: 0:128 and 128:192
BAND = 16                 # output image rows per band
NB = H // BAND            # 8 bands
PWR = BAND + 2            # padded-buffer rows per band (halo)
PBW = 130                 # padded row width (1 + 128 + 1)
PBSZ = PWR * PBW          # padded band cols per chunk
SUB = 512                 # output subtile cols (4 image rows)
NSUB = BAND * W // SUB    # 4 per band
XBR = 17                  # max x rows loaded per band
XBC = XBR * W
TAPS = [(di, dj) for di in range(3) for dj in range(3)]
DVE_TAPS = (3, 5)         # inline-band v0 taps computed on the DVE
DVE_TAPS_DEF = ()         # deferred bands: all taps on the PE
DEFER = 6                 # v-taps of bands >= DEFER run after the middle

_BUILT = {}


def _band_rows(b):
    """pb rows [sr, er) computed this band (rest: halo copy / pad)."""
    sr = 1 if b == 0 else 2
    er = 17 if b == NB - 1 else 18
    return sr, er


def _row_chunks(b):
    sr, er = _band_rows(b)
    out = []
    r = sr
    while r < er:
        nr = min(4, er - r)
        out.append((r, nr))
        r += nr
    return out


def _build(iters=1):
    import concourse.mybir as mybir
    import concourse.tile as tile
    from concourse import bacc

    f32 = mybir.dt.float32
    f16 = mybir.dt.float16
    f8 = mybir.dt.float8e4
    Alu = mybir.AluOpType
    Act = mybir.ActivationFunctionType
    Ax = mybir.AxisListType

    nc = bacc.Bacc(
        "TRN2", target_bir_lowering=False, debug=False,
        enable_asserts=False, num_devices=8,
    )

    # DRAM I/O (per-core shapes)
    xb = nc.dram_tensor("xb", (C, HW), f16, kind="ExternalInput").ap()
    wqv = nc.dram_tensor("wqv", (P0, 768), f16, kind="ExternalInput").ap()
    cp16 = nc.dram_tensor("cp16", (P0, 3200), f16, kind="ExternalInput").ap()
    cp32 = nc.dram_tensor("cp32", (P0, 795), f32, kind="ExternalInput").ap()
    dq8 = nc.dram_tensor("dq8", (P0, 1792), f8, kind="ExternalInput").ap()
    out = nc.dram_tensor("out", (C, HW), f16, kind="ExternalOutput").ap()
    import os
    _abl = set((os.environ.get("KABL") or "").split(","))  # timing ablations

    import contextlib

    with tile.TileContext(nc) as tc:
      with (tc.For_i(0, iters, 1) if iters > 1 else contextlib.nullcontext()):
        with (
            tc.tile_pool(name="const", bufs=1) as cp,
            tc.tile_pool(name="band", bufs=3) as bp,
            tc.tile_pool(name="xb", bufs=2) as xp,
            tc.tile_pool(name="work", bufs=3) as wkp,
            tc.tile_pool(name="qdt", bufs=8) as qp,
            tc.tile_pool(name="psA", bufs=4, space="PSUM") as psA,
            tc.tile_pool(name="psH", bufs=2, space="PSUM") as psH,
        ):
            # ---- constants (coalesced into two pack DMAs) ----
            wqv_sb = cp.tile([P0, 768], f16)
            c16 = cp.tile([P0, 3200], f16)
            c32 = cp.tile([P0, 795], f32)
            dq8_sb = cp.tile([P0, 1792], f8)  # fp8 q consts:
            # [0:768] q0 DR col-pairs | [768:1152] q0 dj=2 diags |
            # [1152:1408] q-pw DR weights | [1408:1792] q1 3-tap DR blocks
            dq_sb = c16[:, 0:1152]
            dv_sb = c16[:, 1152:2304]
            dv1p_sb = c16[:, 2304:2496]       # v1 paired taps
            dv1s_sb = c16[:, 2496:2688]       # v1 single taps (rows 64:128 zero)
            dv1p2_sb = c16[:, 2688:2752]      # v1 row-pair taps (1,2)|(0,2)
            eye_sb = c16[:, 3072:3200]
            wp_sb = c32[:, 0:384]             # WpT rows 0:128 | rows 128:192
            dvw_sb = c32[:, 384:393]          # v0 tap weight columns
            tq_sb = c32[:, 393:395]           # [:,0]=ch0..127, [0:64,1]=ch128..
            bm0_sb = c32[:, 395:587]          # blockdiag mask rows 0:128
            bm1_sb = c32[0:P1, 587:779]       # rows 128:192
            hm0_sb = c32[:, 779:787]          # head-select mask
            hm1_sb = c32[0:P1, 787:795]
            qd_sb = cp.tile([P0, 8192], f16)  # qd: [:,0:4096] | [0:64,4096:8192]
            vdw_sb = cp.tile([P0, 2 * HW], f16)  # v_dw: [:,0:HW] | [64:128,HW:2HW]
            g0a = cp.tile([P0, C], f32)       # gram accumulator rows 0:128
            g1a = cp.tile([P1, C], f32)       # rows 128:192
            srow = cp.tile([P0, C], f32)      # s_d broadcast to all partitions
            wf_sb = cp.tile([P0, 384], f16)   # WfT K0 | [64:128,192:384] K1
            A0 = cp.tile([P0, C], f32)        # blockdiag(attn) rows 0:128
            A1 = cp.tile([P1, C], f32)        # rows 128:192
            ssq = cp.tile([P0, 2], f32)       # row sum-of-squares
            s8 = cp.tile([P0, 16], f32)       # segment-reduce scratch
            rn = cp.tile([P0, 2], f32)        # 1/||q|| * sqrt(temp)
            scr = cp.tile([P0, SUB], f32)     # scratch

            def load_rest_consts():
                nc.sync.dma_start(out=c16[:], in_=cp16[:])
                nc.sync.dma_start(out=c32[:], in_=cp32[:])
                nc.gpsimd.memset(g0a[:], 0.0)
                nc.gpsimd.memset(g1a[:], 0.0)
                # zero-pad so the final k1 contraction can run as K=128
                nc.gpsimd.memset(wf_sb[0:P1, 192:384], 0.0)
                # preload the ACT Sqrt/Exp table so the switch doesn't land
                # on the critical band->middle transition
                nc.vector.memset(scr[0:1, 3:4], 1.0)
                nc.scalar.activation(scr[0:1, 4:5], scr[0:1, 3:4], Act.Sqrt)

            def dma_xband(b, xband, src):
                sr, er = _band_rows(b)
                xlo = b * BAND - 1 + sr
                xhi = b * BAND - 1 + er
                nxc = (xhi - xlo) * W
                if b == 0:
                    # split so the first pw subtile's rows land early
                    cut = 5 * W
                    nc.sync.dma_start(out=xband[:, 0:cut],
                                      in_=src[0:P0, 0:cut])
                    nc.sync.dma_start(out=xband[0:P1, XBC:XBC + cut],
                                      in_=src[P0:C, 0:cut])
                    nc.sync.dma_start(out=xband[:, cut:nxc],
                                      in_=src[0:P0, cut:nxc])
                    nc.sync.dma_start(out=xband[0:P1, XBC + cut:XBC + nxc],
                                      in_=src[P0:C, cut:nxc])
                    return
                nc.sync.dma_start(out=xband[:, 0:nxc],
                                  in_=src[0:P0, xlo * W:xhi * W])
                nc.sync.dma_start(out=xband[0:P1, XBC:XBC + nxc],
                                  in_=src[P0:C, xlo * W:xhi * W])

            # per-band padded buffers, kept across the deferral window
            pb_views = {}   # b -> (pbq, pbv, pbvp, pbqp) flat tiles
            pw_pend = {}    # b -> pending pw stage-copy work
            xbands = {}

            def load_xband(b):
                t = xp.tile([P0, 2 * XBC], f16, tag="xband")
                # zero the unused partitions of the k1 chunk so pw can run
                # K=128 streams with zero-padded data (faster PE config)
                nc.gpsimd.memset(t[P1:P0, XBC:2 * XBC], 0.0)
                dma_xband(b, t, xb)
                xbands[b] = t

            def pw_stage(b):
                xband = xbands.pop(b)
                pbq = bp.tile([P0, PBSZ], f8, tag="pbq")
                pbv = bp.tile([P0, PBSZ], f16, tag="pbv")
                pbvp = bp.tile([P0, PBSZ], f16, tag="pbvp")  # v1: orig 0:64, dup 64:128
                pbvp2 = bp.tile([P0, PBSZ], f16, tag="pbvp2")  # v1: orig 0:64, dup(+row) 64:128
                pbqp = bp.tile([P0, PBSZ], f8, tag="pbqp")  # q1: dup 0:64, orig 64:128
                pbqv = pbq[:].rearrange("p (r c) -> p r c", c=PBW)
                pbvv = pbv[:].rearrange("p (r c) -> p r c", c=PBW)
                pvpv = pbvp[:].rearrange("p (r c) -> p r c", c=PBW)
                pqpv = pbqp[:].rearrange("p (r c) -> p r c", c=PBW)
                # side-column pads (left col always; right col where read)
                for vw in (pbqv, pbvv):
                    nc.gpsimd.memset(vw[:, :, 0:1], 0.0)
                    nc.gpsimd.memset(vw[:, :, 129:130], 0.0)
                nc.gpsimd.memset(pvpv[0:P1, :, 0:1], 0.0)
                nc.gpsimd.memset(pvpv[0:P1, :, 129:130], 0.0)
                nc.gpsimd.memset(pqpv[P1:P0, :, 0:1], 0.0)
                nc.gpsimd.memset(pqpv[P1:P0, :, 129:130], 0.0)
                # top/bottom image pad rows
                if b == 0:
                    nc.gpsimd.memset(pbqv[:, 0, :], 0.0)
                    nc.gpsimd.memset(pbvv[:, 0, :], 0.0)
                    nc.gpsimd.memset(pvpv[0:P1, 0, :], 0.0)
                    nc.gpsimd.memset(pqpv[P1:P0, 0, :], 0.0)
                else:
                    # halo: rows 0:2 = previous band's rows 16:18
                    oq, ov, ovp, _ovp2, oqp = pb_views[b - 1]
                    oqv = oq[:].rearrange("p (r c) -> p r c", c=PBW)
                    ovv = ov[:].rearrange("p (r c) -> p r c", c=PBW)
                    ovpv = ovp[:].rearrange("p (r c) -> p r c", c=PBW)
                    oqpv = oqp[:].rearrange("p (r c) -> p r c", c=PBW)
                    nc.scalar.copy(pbqv[:, 0:2, :], oqv[:, 16:18, :])
                    nc.scalar.copy(pbvv[:, 0:2, :], ovv[:, 16:18, :])
                    nc.vector.tensor_copy(pvpv[0:P1, 0:2, :], ovpv[0:P1, 16:18, :])
                    nc.vector.tensor_copy(pqpv[P1:P0, 0:2, :], oqpv[P1:P0, 16:18, :])
                if b == NB - 1:
                    nc.gpsimd.memset(pbqv[:, PWR - 1, :], 0.0)
                    nc.gpsimd.memset(pbvv[:, PWR - 1, :], 0.0)
                    nc.gpsimd.memset(pvpv[0:P1, PWR - 1, :], 0.0)
                    nc.gpsimd.memset(pqpv[P1:P0, PWR - 1, :], 0.0)
                sr, _er = _band_rows(b)
                tiles = []
                for rs, nr in _row_chunks(b):
                    ncols = nr * W
                    xoff = (rs - sr) * W
                    pq0 = psA.tile([P0, ncols], f32, tag="pw",
                                   padded_shape=[P0, SUB])
                    pv0 = psA.tile([P0, ncols], f32, tag="pw",
                                   padded_shape=[P0, SUB])
                    p1 = psA.tile([P0, ncols], f32, tag="pw",
                                  padded_shape=[P0, SUB])
                    r0 = xband[:, xoff:xoff + ncols]
                    r1 = xband[:, XBC + xoff:XBC + xoff + ncols]
                    if "pw" not in _abl:
                        nc.tensor.matmul(pq0[:], wqv_sb[:, 0:128], r0,
                                         start=True, stop=False)
                        nc.tensor.matmul(pv0[:], wqv_sb[:, 256:384], r0,
                                         start=True, stop=False)
                        nc.tensor.matmul(p1[:], wqv_sb[:, 512:640], r0,
                                         start=True, stop=False)
                        nc.tensor.matmul(pq0[:], wqv_sb[:, 128:256],
                                         r1, start=False, stop=True)
                        nc.tensor.matmul(pv0[:], wqv_sb[:, 384:512],
                                         r1, start=False, stop=True)
                        nc.tensor.matmul(p1[:], wqv_sb[:, 640:768],
                                         r1, start=False, stop=True)
                    else:
                        nc.vector.memset(pq0[:, 0:1], 0.0)
                        nc.vector.memset(pv0[:, 0:1], 0.0)
                        nc.vector.memset(p1[:, 0:1], 0.0)
                    tiles.append((rs, nr, pq0, pv0, p1))
                pw_pend[b] = (tiles, pbqv, pbvv, pvpv, pqpv, pbvp,
                              pbvp2, pbqp)
                pb_views[b] = (pbq, pbv, pbvp, pbvp2, pbqp)

            def pw_copy_stage(b):
                """Stage-copy the pw PSUM results; issued after qtap(b-1)
                so the qd stages aren't stuck behind these in the ACT
                queue."""
                (tiles, pbqv, pbvv, pvpv, pqpv, pbvp, pbvp2,
                 pbqp) = pw_pend.pop(b)
                for rs, nr, pq0, pv0, p1 in tiles:
                    qv0 = pq0[:].rearrange("p (r c) -> p r c", c=W)
                    vv0 = pv0[:].rearrange("p (r c) -> p r c", c=W)
                    vv1 = p1[:].rearrange("p (r c) -> p r c", c=W)
                    nc.scalar.copy(pbqv[:, rs:rs + nr, 1:129], qv0)
                    nc.scalar.copy(pbvv[:, rs:rs + nr, 1:129], vv0)
                    nc.vector.tensor_copy(pvpv[0:P1, rs:rs + nr, 1:129],
                                          vv1[0:P1])
                    nc.vector.tensor_copy(pqpv[P1:P0, rs:rs + nr, 1:129],
                                          vv1[P1:P0])
                # flat-shifted duplicates: dup[f] = orig[f - 1] so a window
                # at dj reads tap (di, dj-1) on the dup partitions.
                nc.sync.dma_start(out=pbvp[P1:P0, 1:PBSZ],
                                  in_=pbvp[0:P1, 0:PBSZ - 1])
                # q1 dup is shifted LEFT (dup[f] = orig[f+1]) so the fp8
                # DoubleRow window can start at an even byte offset
                nc.sync.dma_start(out=pbqp[0:P1, 0:PBSZ - 1],
                                  in_=pbqp[P1:P0, 1:PBSZ])
                # second v1 dup plane: orig on 0:64, row-shifted dup on
                # 64:128 (dup2[f] = orig[f - PBW]) pairs taps (1,2)+(0,2)
                nc.sync.dma_start(out=pbvp2[0:P1, :],
                                  in_=pbvp[0:P1, :])
                nc.sync.dma_start(out=pbvp2[P1:P0, PBW:PBSZ],
                                  in_=pbvp[0:P1, 0:PBSZ - PBW])

            def qtap_stage(b):
                pbq, _pbv, _pbvp, _pbvp2, pbqp = pb_views[b]
                pbqv = pbq[:].rearrange("p (r c) -> p r c", c=PBW)
                pqpv = pbqp[:].rearrange("p (r c) -> p r c", c=PBW)
                pqd0 = psH.tile([P0, SUB], f32, tag="tap0")
                pqd1 = psH.tile([P1, SUB], f32, tag="tap1")
                o0 = pqd0[:].rearrange("p (r c) -> p r c", c=64)
                o1 = pqd1[:].rearrange("p (r c) -> p r c", c=64)
                if "qtap" not in _abl:
                    DRm = mybir.MatmulPerfMode.DoubleRow
                    # fp8 DoubleRow col-pairs (di,0)+(di,1): k-tile dim is
                    # the low bit of the padded-buffer column
                    for di in range(3):
                        rhs = pbqv[:, di:di + BAND:2, 0:128].rearrange(
                            "p r (c s) -> p s r c", s=2)
                        lhs = dq8_sb[:, di * 256:(di + 1) * 256].rearrange(
                            "p (s m) -> p s m", s=2)
                        nc.tensor.matmul(o0, lhs, rhs, start=(di == 0),
                                         stop=False, perf_mode=DRm)
                    # dj=2 taps as plain fp8 streams
                    for di in range(3):
                        rhs0 = pbqv[:, di:di + BAND:2, 2:2 + W:2]
                        nc.tensor.matmul(
                            o0, dq8_sb[:, 768 + di * 128:896 + di * 128],
                            rhs0, start=False, stop=(di == 2))
                    # q1: 3 fp8 DoubleRow streams, 3 taps each: partition
                    # stacking (dup|orig) x k-tile windows; dup is LEFT
                    # shifted so the window base stays even: k0 = {(di,1)
                    # dup, (di,0) orig}, k1 = {(di,2) dup, zero orig}
                    for di in range(3):
                        rhs = pqpv[:, di:di + BAND:2, 0:128].rearrange(
                            "p r (c s) -> p s r c", s=2)
                        lhs = dq8_sb[:, 1408 + di * P0:1408 + (di + 1) * P0
                                     ].rearrange("p (s m) -> p s m", s=2)
                        nc.tensor.matmul(o1, lhs, rhs, start=(di == 0),
                                         stop=(di == 2), perf_mode=DRm)
                else:
                    nc.vector.memset(pqd0[:, 0:1], 0.0)
                    nc.vector.memset(pqd1[:, 0:1], 0.0)
                nc.scalar.copy(qd_sb[:, b * SUB:(b + 1) * SUB], pqd0[:])
                nc.scalar.copy(qd_sb[0:P1, 4096 + b * SUB:4096 + (b + 1) * SUB],
                               pqd1[:])

            def vtap_stage(b, deferred=False):
                _pbq, pbv, pbvp, pbvp2, _pbqp = pb_views[b]
                pbvv = pbv[:].rearrange("p (r c) -> p r c", c=PBW)
                pvpv = pbvp[:].rearrange("p (r c) -> p r c", c=PBW)
                pvp2v = pbvp2[:].rearrange("p (r c) -> p r c", c=PBW)
                dve_taps = DVE_TAPS_DEF if deferred else DVE_TAPS
                h0 = b * BAND
                # zero rows 0:64 of this band's v1 plane so the final k1
                # contraction can run as K=128 over zero-padded data
                nc.gpsimd.memset(
                    vdw_sb[0:P1, HW + h0 * W:HW + (h0 + BAND) * W], 0.0)
                for s in range(NSUB):
                    if s % 2 == 0:
                        ptv0 = psH.tile([P0, SUB], f32, tag="tap0")
                        ptv1f = psH.tile([P0, SUB], f32, tag="tap1")
                    else:
                        ptv0 = psA.tile([P0, SUB], f32, tag="pw")
                        ptv1f = psA.tile([P0, SUB], f32, tag="pw")
                    ptv1 = ptv1f[P1:P0, :]
                    ov0 = ptv0[:].rearrange("p (r c) -> p r c", c=W)
                    ov1 = ptv1.rearrange("p (r c) -> p r c", c=W)
                    if "vtap" not in _abl:
                        for t, (di, dj) in enumerate(TAPS):
                            if t in dve_taps:
                                continue
                            st = (t == 0)
                            sp = (t == 8)
                            rhs0 = pbvv[:, 4 * s + di:4 * s + di + 4, dj:dj + W]
                            nc.tensor.matmul(ov0,
                                             dv_sb[:, t * P0:(t + 1) * P0],
                                             rhs0, start=st, stop=sp)
                        for di in range(3):
                            # paired: orig parts 0:64 -> tap (di,1); dup -> (di,0)
                            rhp = pvpv[:, 4 * s + di:4 * s + di + 4, 1:1 + W]
                            nc.tensor.matmul(
                                ov1, dv1p_sb[:, di * P1:(di + 1) * P1], rhp,
                                start=(di == 0), stop=False,
                                tile_position=(0, P1))
                        # row-pair: pbvp2 orig -> tap (1,2); dup2 -> (0,2)
                        rhp2 = pvp2v[:, 4 * s + 1:4 * s + 5, 2:2 + W]
                        nc.tensor.matmul(
                            ov1, dv1p2_sb[:], rhp2,
                            start=False, stop=False,
                            tile_position=(0, P1))
                        # single: orig parts 0:64 -> tap (2,2); dup parts
                        # ride along with zero weights (K=128 config)
                        rhs = pvpv[:, 4 * s + 2:4 * s + 6, 2:2 + W]
                        nc.tensor.matmul(
                            ov1, dv1s_sb[:, 2 * P1:3 * P1], rhs,
                            start=False, stop=True,
                            tile_position=(0, P1))
                    else:
                        nc.vector.memset(ptv0[:, 0:1], 0.0)
                        nc.vector.memset(ptv1[:, 0:1], 0.0)
                    cs = h0 * W + s * SUB
                    nc.vector.tensor_copy(vdw_sb[:, cs:cs + SUB], ptv0[:])
                    nc.scalar.copy(vdw_sb[P1:P0, HW + cs:HW + cs + SUB], ptv1)
                    if "vtap" not in _abl and deferred:
                        # per-subtile DVE taps right after this subtile's cast
                        avs = vdw_sb[:, cs:cs + SUB].rearrange(
                            "p (r c) -> p r c", c=W)
                        for t in dve_taps:
                            di, dj = TAPS[t]
                            rhs = pbvv[:, 4 * s + di:4 * s + di + 4, dj:dj + W]
                            nc.vector.scalar_tensor_tensor(
                                avs, rhs, dvw_sb[:, t:t + 1], avs,
                                Alu.mult, Alu.add)
                # DVE-side v0 taps accumulate onto the staged band
                if "vtap" not in _abl and not deferred:
                    bw = h0 * W
                    av = vdw_sb[:, bw:bw + BAND * W].rearrange(
                        "p (r c) -> p r c", c=W)
                    for t in dve_taps:
                        di, dj = TAPS[t]
                        rhs = pbvv[:, di:di + BAND, dj:dj + W]
                        nc.vector.scalar_tensor_tensor(
                            av, rhs, dvw_sb[:, t:t + 1], av,
                            Alu.mult, Alu.add)

            qdT_tiles = {}

            def trans_stage(b):
                tiles = []
                pts = []
                for kb in range(4):
                    kcol = b * SUB + kb * P0
                    pt0 = psA.tile([P0, P0], f16, tag="pw",
                                   padded_shape=[P0, SUB])
                    nc.tensor.transpose(pt0[:], qd_sb[:, kcol:kcol + P0],
                                        eye_sb[:])
                    pts.append(pt0)
                for kb in range(4):
                    kcol = b * SUB + kb * P0
                    pt1 = psA.tile([P0, P1], f16, tag="pw",
                                   padded_shape=[P0, SUB])
                    nc.tensor.transpose(pt1[:],
                                        qd_sb[0:P1, 4096 + kcol:4096 + kcol + P0],
                                        eye_sb[0:P1, 0:P1])
                    qdT = qp.tile([P0, C], f16, tag="qdT")
                    nc.vector.tensor_copy(qdT[:, 0:P0], pts[kb][:])
                    nc.vector.tensor_copy(qdT[:, P0:C], pt1[:])
                    tiles.append(qdT)
                qdT_tiles[b] = tiles

            def gramm_stage(b):
                pgb0 = psH.tile([P0, C], f32, tag="tap0")
                pgb1 = psH.tile([P1, C], f32, tag="tap1")
                if "gram" in _abl:
                    nc.vector.memset(pgb0[:, 0:1], 0.0)
                    nc.vector.memset(pgb1[:, 0:1], 0.0)
                else:
                    tiles = qdT_tiles.pop(b)
                    for kb in range(4):
                        nc.tensor.matmul(pgb0[:], tiles[kb][:, 0:P0],
                                         tiles[kb][:],
                                         start=(kb == 0), stop=(kb == 3))
                    for kb in range(4):
                        nc.tensor.matmul(pgb1[:], tiles[kb][:, P0:C],
                                         tiles[kb][:],
                                         start=(kb == 0), stop=(kb == 3))
                nc.vector.tensor_tensor(g0a[:], g0a[:], pgb0[:], Alu.add)
                nc.vector.tensor_tensor(g1a[:], g1a[:], pgb1[:], Alu.add)

            def middle():
                # ---- row scales: rn = sqrt(temp) / ||qd_row|| ----
                # ||qd_row||^2 = diag(G). ACT Sqrt is low-precision (~4e-3);
                # one Newton step on y=sqrt(ss): y' = 0.5*(y + ss/y).
                nc.vector.tensor_tensor(scr[:, 0:P0], g0a[:, 0:P0], eye_sb[:],
                                        Alu.mult)
                nc.vector.tensor_reduce(ssq[:, 0:1], scr[:, 0:P0], Ax.X,
                                        Alu.add)
                nc.vector.tensor_tensor(scr[0:P1, 0:P1], g1a[0:P1, P0:C],
                                        eye_sb[0:P1, 0:P1], Alu.mult)
                nc.vector.tensor_reduce(ssq[0:P1, 1:2], scr[0:P1, 0:P1],
                                        Ax.X, Alu.add)
                for ss_ap, rn_ap, tq_ap in (
                    (ssq[:, 0:1], rn[:, 0:1], tq_sb[:, 0:1]),
                    (ssq[0:P1, 1:2], rn[0:P1, 1:2], tq_sb[0:P1, 1:2]),
                ):
                    y = scr[0:ss_ap.shape[0], 0:1]
                    yr = scr[0:ss_ap.shape[0], 1:2]
                    nc.scalar.activation(y, ss_ap, Act.Sqrt)
                    nc.vector.reciprocal(yr, y)                       # 1/y
                    nc.vector.tensor_tensor(yr, yr, ss_ap, Alu.mult)  # ss/y
                    nc.vector.tensor_tensor(y, y, yr, Alu.add)
                    nc.vector.tensor_scalar_mul(y, y, 0.5)            # sqrt
                    nc.vector.reciprocal(rn_ap, y)
                    nc.vector.tensor_tensor(rn_ap, rn_ap, tq_ap, Alu.mult)

                # logits = diag(s) G diag(s): row scale by s_c, then
                # elementwise multiply by s_d replicated across partitions.
                nc.sync.dma_start(out=srow[0:1, 0:P0], in_=rn[:, 0:1])
                nc.sync.dma_start(out=srow[0:1, P0:C], in_=rn[0:P1, 1:2])
                nc.gpsimd.partition_broadcast(srow[:], srow[0:1, :])
                nc.vector.tensor_scalar_mul(g0a[:], g0a[:], rn[:, 0:1])
                nc.vector.tensor_scalar_mul(g1a[:], g1a[:], rn[0:P1, 1:2])
                nc.vector.tensor_tensor(g0a[:], g0a[:], srow[:], Alu.mult)
                nc.vector.tensor_tensor(g1a[:], g1a[:], srow[0:P1, :],
                                        Alu.mult)

                # ---- masked softmax over the diagonal 24-blocks ----
                for g, hm, bm, A, npd in (
                    (g0a, hm0_sb, bm0_sb, A0, P0),
                    (g1a, hm1_sb, bm1_sb, A1, P1),
                ):
                    gv = g[0:npd, :].rearrange("p (h c) -> p h c", c=CHD)
                    nc.vector.tensor_reduce(s8[0:npd, 0:8], gv, Ax.X, Alu.max)
                    nc.vector.tensor_tensor(s8[0:npd, 8:16], s8[0:npd, 0:8],
                                            hm[0:npd, :], Alu.mult)
                    nc.vector.tensor_reduce(scr[0:npd, 0:1], s8[0:npd, 8:16],
                                            Ax.X, Alu.add)
                    nc.vector.tensor_scalar_sub(g[0:npd, :], g[0:npd, :],
                                                scr[0:npd, 0:1])
                    nc.scalar.activation(g[0:npd, :], g[0:npd, :], Act.Exp)
                    nc.vector.tensor_reduce(s8[0:npd, 0:8], gv, Ax.X, Alu.add)
                    nc.vector.tensor_tensor(s8[0:npd, 8:16], s8[0:npd, 0:8],
                                            hm[0:npd, :], Alu.mult)
                    nc.vector.tensor_reduce(scr[0:npd, 1:2], s8[0:npd, 8:16],
                                            Ax.X, Alu.add)
                    nc.vector.reciprocal(scr[0:npd, 2:3], scr[0:npd, 1:2])
                    nc.vector.scalar_tensor_tensor(
                        A[0:npd, :], g[0:npd, :], scr[0:npd, 2:3],
                        bm[0:npd, :], Alu.mult, Alu.mult)

            def wf_and_final():
                # WfT = (Wp @ blockdiag(attn))^T contraction
                pwf0 = psH.tile([P0, C], f32, tag="tap0")
                pwf1f = psH.tile([P0, C], f32, tag="tap1")
                nc.tensor.matmul(pwf0[:], A0[:, 0:P0], wp_sb[:, 0:192],
                                 start=True, stop=False)
                nc.tensor.matmul(pwf0[:], A1[:, 0:P0], wp_sb[0:P1, 192:384],
                                 start=False, stop=True)
                nc.tensor.matmul(pwf1f[P1:P0, :], A0[:, P0:C], wp_sb[:, 0:192],
                                 start=True, stop=False)
                nc.tensor.matmul(pwf1f[P1:P0, :], A1[:, P0:C],
                                 wp_sb[0:P1, 192:384],
                                 start=False, stop=True)
                nc.scalar.copy(wf_sb[:, 0:192], pwf0[:])
                nc.scalar.copy(wf_sb[P1:P0, 192:384], pwf1f[P1:P0, :])

                # ==== final sweep: out = WfT-contraction @ v_dw ====
                for i in range(HW // SUB):
                    if i % 2 == 0:
                        po0 = psH.tile([P0, SUB], f32, tag="tap0")
                        po1f = psH.tile([P0, SUB], f32, tag="tap1")
                    else:
                        po0 = psA.tile([P0, SUB], f32, tag="pw")
                        po1f = psA.tile([P0, SUB], f32, tag="pw")
                    po1 = po1f[0:P1, :]
                    r0 = vdw_sb[:, i * SUB:(i + 1) * SUB]
                    r1 = vdw_sb[:, HW + i * SUB:HW + (i + 1) * SUB]
                    if "final" not in _abl:
                        # k1 runs K=128 with zero-padded rows 0:64
                        nc.tensor.matmul(po0[:], wf_sb[:, 0:P0], r0,
                                         start=True, stop=False)
                        nc.tensor.matmul(po0[:], wf_sb[:, 192:320], r1,
                                         start=False, stop=True)
                        nc.tensor.matmul(po1, wf_sb[:, P0:192], r0,
                                         start=True, stop=False)
                        nc.tensor.matmul(po1, wf_sb[:, 320:384], r1,
                                         start=False, stop=True)
                    else:
                        nc.vector.memset(po0[:, 0:1], 0.0)
                        nc.vector.memset(po1[:, 0:1], 0.0)
                    ost0 = wkp.tile([P0, SUB], f16, tag="ost0")
                    ost1 = wkp.tile([P1, SUB], f16, tag="ost1")
                    nc.scalar.copy(ost0[:], po0[:])
                    nc.vector.tensor_copy(ost1[:], po1)
                    nc.sync.dma_start(out=out[0:P0, i * SUB:(i + 1) * SUB],
                                      in_=ost0[:])
                    nc.sync.dma_start(out=out[P0:C, i * SUB:(i + 1) * SUB],
                                      in_=ost1[:])

            # ========== schedule ==========
            nc.sync.dma_start(out=wqv_sb[:, 0:640], in_=wqv[:, 0:640])
            nc.sync.dma_start(out=wqv_sb[0:P1, 640:768], in_=wqv[0:P1, 640:768])
            # dq8 must be issued before its first reader (qtap at b=1)
            nc.sync.dma_start(out=dq8_sb[:], in_=dq8[:])
            load_xband(0)
            for b in range(NB):
                if b + 1 < NB:
                    load_xband(b + 1)
                pw_stage(b)
                if b == 0:
                    load_rest_consts()
                if b >= 1:
                    qtap_stage(b - 1)
                pw_copy_stage(b)
                if b >= 2:
                    gramm_stage(b - 2)
                if b >= 1:
                    trans_stage(b - 1)
                if b >= 2 and b - 2 < DEFER:
                    vtap_stage(b - 2)
            qtap_stage(NB - 1)
            gramm_stage(NB - 2)
            trans_stage(NB - 1)
            vtap_stage(DEFER, deferred=True)
            gramm_stage(NB - 1)
            middle()
            for b in range(DEFER + 1, NB):
                vtap_stage(b, deferred=True)
            wf_and_final()

    nc.compile()
    return nc


def _host_inputs(x, w_qkv, w_dw, w_proj, temperature):
    """Per-core input maps (host-side precompute of all weight transforms)."""
    f = np.float32
    W_q = w_qkv[0:C].astype(f)           # (192,192) out x in
    W_v = w_qkv[2 * C:3 * C].astype(f)
    wq_d = w_dw[0:C, 0].reshape(C, 9).astype(f)        # (192,9) taps (di,dj)
    wv_d = w_dw[2 * C:3 * C, 0].reshape(C, 9).astype(f)

    WqT = W_q.T.astype(f)                # (in, out)
    WvT = W_v.T.astype(f)
    wqv = np.zeros((P0, 768), f)
    wqv[:, 0:128] = WqT[0:P0, 0:128]
    wqv[0:P1, 128:256] = WqT[P0:C, 0:128]
    wqv[:, 256:384] = WvT[0:P0, 0:128]
    wqv[0:P1, 384:512] = WvT[P0:C, 0:128]
    # chunk1 pointwise output order: [v1 (parts 0:64); q1 (parts 64:128)]
    wqv[:, 512:576] = WvT[0:P0, 128:192]
    wqv[:, 576:640] = WqT[0:P0, 128:192]
    wqv[0:P1, 640:704] = WvT[P0:C, 128:192]
    wqv[0:P1, 704:768] = WqT[P0:C, 128:192]

    def pack_diag(wd, lo, n):
        outm = np.zeros((n, 9 * n), f)
        for t in range(9):
            np.fill_diagonal(outm[:, t * n:(t + 1) * n], wd[lo:lo + n, t])
        return outm

    wp_pack = np.zeros((P0, 384), f)
    WpT = w_proj.T.astype(f)
    wp_pack[:, 0:192] = WpT[0:P0]
    wp_pack[0:P1, 192:384] = WpT[P0:C]

    # chunk1 paired/single tap diagonals.
    # v1 buffer: orig at parts 0:64 (window dj=1 -> tap (di,1)),
    #            dup  at parts 64:128 (window dj=1 -> tap (di,0)).
    dv1p_m = np.zeros((P0, 3 * P1), f)
    dv1s_m = np.zeros((P1, 3 * P1), f)
    # q1 buffer: dup at parts 0:64 (-> tap (di,0)), orig at 64:128 (-> (di,1))
    dq1p_m = np.zeros((P0, 3 * P1), f)
    dq1s_m = np.zeros((P0, 3 * P1), f)
    for di in range(3):
        sl = slice(di * P1, (di + 1) * P1)
        np.fill_diagonal(dv1p_m[0:P1, sl], wv_d[P0:C, 3 * di + 1])
        np.fill_diagonal(dv1p_m[P1:P0, sl], wv_d[P0:C, 3 * di + 0])
        np.fill_diagonal(dv1s_m[:, sl], wv_d[P0:C, 3 * di + 2])
        np.fill_diagonal(dq1p_m[0:P1, sl], wq_d[P0:C, 3 * di + 0])
        np.fill_diagonal(dq1p_m[P1:P0, sl], wq_d[P0:C, 3 * di + 1])
        np.fill_diagonal(dq1s_m[P1:P0, sl], wq_d[P0:C, 3 * di + 2])

    tqv = np.sqrt(np.repeat(temperature.reshape(HEADS).astype(f),
                            CHD)).reshape(C, 1)
    eye = np.eye(P0, dtype=np.float16)

    heads0 = np.arange(P0) // CHD
    heads1 = (P0 + np.arange(P1)) // CHD
    headc = np.arange(C) // CHD
    bm0_m = (headc[None, :] == heads0[:, None]).astype(f)
    bm1_m = (headc[None, :] == heads1[:, None]).astype(f)
    hm0_m = (np.arange(HEADS)[None, :] == heads0[:, None]).astype(f)
    hm1_m = (np.arange(HEADS)[None, :] == heads1[:, None]).astype(f)

    h = np.float16
    # f16 const pack: dq | dv | dv1p | dv1s | dq1p | dq1s | eye
    c16 = np.zeros((P0, 3200), h)
    c16[:, 0:1152] = pack_diag(wq_d, 0, P0).astype(h)
    c16[:, 1152:2304] = pack_diag(wv_d, 0, P0).astype(h)
    c16[:, 2304:2496] = dv1p_m.astype(h)
    c16[0:P1, 2496:2688] = dv1s_m.astype(h)
    dv1p2_m = np.zeros((P0, P1), f)
    np.fill_diagonal(dv1p2_m[0:P1, :], wv_d[P0:C, 5])
    np.fill_diagonal(dv1p2_m[P1:P0, :], wv_d[P0:C, 2])
    c16[:, 2688:2752] = dv1p2_m.astype(h)
    c16[:, 3072:3200] = eye
    # f32 const pack: wp | dvw | tq | bm0 | bm1 | hm0 | hm1
    c32 = np.zeros((P0, 795), f)
    c32[:, 0:384] = wp_pack
    c32[:, 384:393] = wv_d[0:P0]
    c32[:, 393:394] = tqv[0:P0]
    c32[0:P1, 394:395] = tqv[P0:C]
    c32[:, 395:587] = bm0_m
    c32[0:P1, 587:779] = bm1_m
    c32[:, 779:787] = hm0_m
    c32[0:P1, 787:795] = hm1_m
    # fp8 q-path constants:
    #  [0:768]     q0 DoubleRow col-pairs [(di,0)|(di,1)]
    #  [768:1152]  q0 dj=2 taps as plain diagonals
    #  [1152:1408] q-pointwise DR weights (k-tile1 zero-padded)
    #  [1408:1792] q1 3-tap DR blocks (dup|orig stacking x dj=1/2 windows)
    import ml_dtypes
    dq8_m = np.zeros((P0, 1792), f)
    for di in range(3):
        np.fill_diagonal(dq8_m[:, di * 256:di * 256 + 128],
                         wq_d[0:P0, 3 * di + 0])
        np.fill_diagonal(dq8_m[:, di * 256 + 128:(di + 1) * 256],
                         wq_d[0:P0, 3 * di + 1])
        np.fill_diagonal(dq8_m[:, 768 + di * 128:896 + di * 128],
                         wq_d[0:P0, 3 * di + 2])
    dq8_m[:, 1152:1280] = WqT[0:P0, 0:128]
    dq8_m[0:P1, 1280:1408] = WqT[P0:C, 0:128]
    for di in range(3):
        base = 1408 + di * 128
        np.fill_diagonal(dq8_m[0:P1, base:base + 64],
                         wq_d[P0:C, 3 * di + 1])
        np.fill_diagonal(dq8_m[P1:P0, base:base + 64],
                         wq_d[P0:C, 3 * di + 0])
        np.fill_diagonal(dq8_m[0:P1, base + 64:base + 128],
                         wq_d[P0:C, 3 * di + 2])
    shared = {
        "wqv": wqv.astype(h), "cp16": c16, "cp32": c32,
        "dq8": dq8_m.astype(ml_dtypes.float8_e4m3fn),
    }
    maps = []
    for b in range(8):
        m = dict(shared)
        xf = x[b].reshape(C, HW)
        m["xb"] = np.ascontiguousarray(xf.astype(h))
        maps.append(m)
    return maps


def kernel(x, w_qkv, w_dw, w_proj, temperature, _trace=False, _iters=1):
    from concourse.bass_utils import run_bass_kernel_spmd
    if _iters not in _BUILT:
        _BUILT[_iters] = _build(_iters)
    nc = _BUILT[_iters]
    in_maps = _host_inputs(
        np.asarray(x), np.asarray(w_qkv), np.asarray(w_dw),
        np.asarray(w_proj), np.asarray(temperature))
    res = run_bass_kernel_spmd(nc, in_maps, list(range(8)), trace=_trace)
    outs = [res.results[i]["out"].reshape(C, H, W) for i in range(8)]
    y = np.stack(outs, axis=0).astype(np.float32)
    kernel.last_result = res
    return y
